# revision 1
# baseline (speedup 1.0000x reference)
"""Trainium2 kernel for nn_LocalLatentsTransformer (dense transformer w/ pair
tri-mult updates), distributed across 8 NeuronCores.

Strategy (per sharding hint): sequence-dim sharding of the [n,n,c] pair tensor
and the token stream across the 8 cores; the tri-mult einsums contract across
the sharded axis (XLA GSPMD inserts the all-gathers along the contracted k
axis). Compiled once and cached; subsequent calls reuse the executable.
"""
import numpy as np

NLAYERS = 6
D = 512
P = 128
H = 16
HD = D // H
DC = 256
LAT = 8
EVERY = 2
FS = 64
FP = 32
OPC = 32
NCORES = 8

_CACHE = {}


def _forward_fn():
    import jax
    import jax.numpy as jnp

    def _ln(x, g, b):
        mu = jnp.mean(x, -1, keepdims=True)
        var = jnp.var(x, -1, keepdims=True)
        return (x - mu) * jax.lax.rsqrt(var + 1e-5) * g + b

    def _transition(x, p, mask):
        h = _ln(x, p['ln_g'], p['ln_b'])
        out = (jax.nn.silu(h @ p['W1']) * (h @ p['W2'])) @ p['Wo']
        return (x + out) * mask[..., None]

    def _attn_block(x, pair, c, mask, p):
        b, n, _ = x.shape
        mod = c @ p['cond_W']
        sh1, sc1, g1, sh2, sc2, g2 = jnp.split(mod, 6, axis=-1)
        h = _ln(x, p['ln1_g'], p['ln1_b']) * (1.0 + sc1) + sh1
        q = (h @ p['Wq']).reshape(b, n, H, HD)
        k = (h @ p['Wk']).reshape(b, n, H, HD)
        v = (h @ p['Wv']).reshape(b, n, H, HD)

        def headln(t, g):
            mu = jnp.mean(t, -1, keepdims=True)
            var = jnp.var(t, -1, keepdims=True)
            return (t - mu) * jax.lax.rsqrt(var + 1e-5) * g

        q = headln(q, p['qln_g'])
        k = headln(k, p['kln_g'])
        bias = _ln(pair, p['pair_ln_g'], p['pair_ln_b']) @ p['Wbias']
        bias = jnp.transpose(bias, (0, 3, 1, 2))
        scores = jnp.einsum('bqhd,bkhd->bhqk', q, k) / np.float32(np.sqrt(HD)) + bias
        scores = jnp.where(mask[:, None, None, :], scores, jnp.float32(-1e9))
        attn = jax.nn.softmax(scores, axis=-1)
        o = jnp.einsum('bhqk,bkhd->bqhd', attn, v).reshape(b, n, D) @ p['Wo']
        x = (x + g1 * o) * mask[..., None]
        h2 = _ln(x, p['ln2_g'], p['ln2_b']) * (1.0 + sc2) + sh2
        ff = jax.nn.gelu(h2 @ p['W1']) @ p['W2']
        x = (x + g2 * ff) * mask[..., None]
        return x

    def _tri_mult(pair, mask2d, p, incoming):
        z = _ln(pair, p['ln_g'], p['ln_b'])
        a = jax.nn.sigmoid(z @ p['Wag']) * (z @ p['Wa']) * mask2d
        bb = jax.nn.sigmoid(z @ p['Wbg']) * (z @ p['Wb']) * mask2d
        if incoming:
            t = jnp.einsum('bkic,bkjc->bijc', a, bb)
        else:
            t = jnp.einsum('bikc,bjkc->bijc', a, bb)
        g = jax.nn.sigmoid(z @ p['Wog'])
        return pair + g * (_ln(t, p['ln2_g'], p['ln2_b']) @ p['Wo'])

    def _pair_update(seqs, pair, mask, p):
        mask2d = (mask[:, :, None] & mask[:, None, :])[..., None].astype(pair.dtype)
        s = _ln(seqs, p['seq_ln_g'], p['seq_ln_b'])
        a = s @ p['Wa_op']
        b_ = s @ p['Wb_op']
        outer = a[:, :, None, :] * b_[:, None, :, :]
        pair = pair + (outer @ p['Wo_op']) * mask2d
        pair = _tri_mult(pair, mask2d, p['tri_out'], incoming=False)
        pair = _tri_mult(pair, mask2d, p['tri_in'], incoming=True)
        return pair

    def forward(x_feats, c_feats, pair_feats, mask, params):
        c = c_feats @ params['cond_in_W']
        c = _transition(_transition(c, params['trans_c1'], mask),
                        params['trans_c2'], mask)
        seqs = (x_feats @ params['seq_in_W']) * mask[..., None]
        pair = pair_feats @ params['pair_in_W']
        for i in range(NLAYERS):
            seqs = _attn_block(seqs, pair, c, mask, params['layers'][i])
            if i < NLAYERS - 1 and params['pair_updates'][i] is not None:
                pair = _pair_update(seqs, pair, mask, params['pair_updates'][i])
        mf = mask[..., None]
        lat = (_ln(seqs, params['lat_ln_g'], params['lat_ln_b']) @ params['lat_W']) * mf
        ca = (_ln(seqs, params['ca_ln_g'], params['ca_ln_b']) @ params['ca_W']) * mf
        return ca, lat

    return forward


def _build(params_tree_def_key):
    import jax
    from jax.sharding import Mesh, PartitionSpec, NamedSharding

    forward = _forward_fn()
    devices = jax.devices()
    result = {}

    if len(devices) >= NCORES:
        try:
            mesh = Mesh(np.asarray(devices[:NCORES]), ("x",))
            repl = NamedSharding(mesh, PartitionSpec())
            row = NamedSharding(mesh, PartitionSpec(None, "x"))

            def sharded_fwd(x_feats, c_feats, pair_feats, mask, params):
                pair_feats = jax.lax.with_sharding_constraint(pair_feats, row)
                ca, lat = forward(x_feats, c_feats, pair_feats, mask, params)
                ca = jax.lax.with_sharding_constraint(ca, repl)
                lat = jax.lax.with_sharding_constraint(lat, repl)
                return ca, lat

            jfn = jax.jit(
                sharded_fwd,
                in_shardings=(repl, repl, row, repl, repl),
                out_shardings=(repl, repl),
            )
            result["fn"] = jfn
            result["mode"] = "gspmd8"
            return result
        except Exception:
            pass

    # Fallback: single-device jit
    result["fn"] = jax.jit(forward)
    result["mode"] = "single"
    return result


def kernel(x_feats, c_feats, pair_feats, mask, params):
    import jax

    key = "k"
    if key not in _CACHE:
        _CACHE[key] = _build(key)
    built = _CACHE[key]
    fn = built["fn"]
    try:
        ca, lat = fn(x_feats, c_feats, pair_feats, mask, params)
        ca = np.asarray(jax.device_get(ca))
        lat = np.asarray(jax.device_get(lat))
    except Exception:
        # robust fallback: single device
        forward = _forward_fn()
        fn1 = jax.jit(forward)
        ca, lat = fn1(x_feats, c_feats, pair_feats, mask, params)
        ca = np.asarray(jax.device_get(ca))
        lat = np.asarray(jax.device_get(lat))
        _CACHE[key] = {"fn": fn1, "mode": "single"}
    return ca, lat
